# revision 10
# baseline (speedup 1.0000x reference)
"""BitNet decoder MLP on 8 Trainium2 NeuronCores (Bass/Tile).

Strategy: data-parallel over batch (512 rows/core). Weights are ternary-quantized
cooperatively (each core quantizes 1/8 of all weight chunks, shared via per-layer
AllGather in bf16). All matmul arithmetic is exact: activations are int8-valued
bf16, weights are {-1,0,1} bf16, accumulation fp32 in PSUM. Per-row dequant
scales fold into the PSUM-eviction pass on the scalar engine; LayerNorm+SiLU run
as fused scalar-engine passes; rsqrt via Newton on the vector engine; rounding
via the fp32 magic-number trick (round-half-even, matches jnp.round).
"""

import numpy as np

import concourse.bass as bass
import concourse.mybir as mybir
import concourse.tile as tile
from concourse import bacc
from concourse.bass_utils import run_bass_kernel_spmd

F32 = mybir.dt.float32
BF16 = mybir.dt.bfloat16
I16 = mybir.dt.int16
AF = mybir.ActivationFunctionType
OP = mybir.AluOpType

N_CORES = 8
P = 128
OBW = 512            # output block width (one PSUM bank of fp32)
MAGIC = 12582912.0   # 1.5 * 2**23: fp32 round-to-nearest-even trick
EPS = 1e-5
RSQRT_MAGIC_HI16 = 0x5F37  # high 16 bits of 0x5f3759df

FULL_CFG = dict(B=4096, D0=1024, H=4096, OBINS=1000)


def _plan(cfg):
    """Static per-layer plan. Returns list of layer dicts and totals."""
    B, D0, H, OBINS = cfg["B"], cfg["D0"], cfg["H"], cfg["OBINS"]
    o3_real = 2 * OBINS
    o3_pad = ((o3_real + OBW - 1) // OBW) * OBW
    dims = [
        dict(din=D0, dout=H, dreal=H),
        dict(din=H, dout=H, dreal=H),
        dict(din=H, dout=H, dreal=H),
        dict(din=H, dout=o3_pad, dreal=o3_real),
    ]
    numels = [H * D0, H * H, H * H, o3_real * H]  # real numels for mean|W|
    layers = []
    base = 0
    for li, d in enumerate(dims):
        n_ic = d["din"] // P
        n_ob = d["dout"] // OBW
        n_ch = n_ob * n_ic
        assert n_ch % N_CORES == 0, (li, n_ch)
        panel_ic = min(16, n_ic)
        assert n_ic % panel_ic == 0
        layers.append(dict(
            li=li, din=d["din"], dout=d["dout"], dreal=d["dreal"],
            n_ic=n_ic, n_ob=n_ob, n_ch=n_ch, per_rank=n_ch // N_CORES,
            panel_ic=panel_ic, n_panels=n_ic // panel_ic,
            numel=numels[li], ch_base=base,
            ob_w=[min(OBW, d["dreal"] - ob * OBW) for ob in range(n_ob)],
        ))
        base += n_ch
    total_ch = base
    assert total_ch % N_CORES == 0
    per_rank = total_ch // N_CORES
    b_core = B // N_CORES
    assert b_core % P == 0
    return layers, total_ch, per_rank, b_core // P


def _rank_chunks(layers, r):
    """List of (layer, g) chunk ids owned by rank r, in shard order."""
    out = []
    for L in layers:
        pr = L["per_rank"]
        for g in range(r * pr, (r + 1) * pr):
            out.append((L["li"], g))
    return out


def _runs(seq, maxlen=8):
    """Group a contiguous range into runs of up to maxlen."""
    out = []
    i = 0
    while i < len(seq):
        out.append(seq[i:i + maxlen])
        i += maxlen
    return out


def _rsqrt_newton(nc, pool, v, n_iter=3):
    """istd = 1/sqrt(v) for v [128,1] fp32 (v > 0), pure-DVE Newton iteration.

    Seed via the fp32 bit trick applied to the high 16 bits only (keeps all
    integer arithmetic exactly representable in the DVE's fp32 ALU)."""
    seed = pool.tile([P, 1], F32, tag="rs_seed")
    # seed_bits = 0x5f370000 - bits(v)/2, all in aligned 32-bit ops; the fp32
    # mantissa noise on the >2^24 intermediate is irrelevant for a Newton seed
    seed_i32 = seed[:].bitcast(mybir.dt.int32)
    v_i32 = v.bitcast(mybir.dt.int32)
    nc.vector.tensor_scalar(seed_i32[:], v_i32[:], -0.5,
                            float(0x5F370000), OP.mult, OP.add)
    y = seed
    t1 = pool.tile([P, 1], F32, tag="rs_t1")
    t2 = pool.tile([P, 1], F32, tag="rs_t2")
    for _ in range(n_iter):
        nc.vector.tensor_tensor(t1[:], y[:], y[:], OP.mult)
        nc.vector.tensor_tensor(t2[:], t1[:], v, OP.mult)
        nc.vector.tensor_scalar(t1[:], t2[:], -0.5, 1.5, OP.mult, OP.add)
        nc.vector.tensor_tensor(y[:], y[:], t1[:], OP.mult)
    return y


def build(cfg, kstop=None):
    import os
    kstop = kstop if kstop is not None else os.environ.get("KSTOP", "")
    layers, total_ch, per_rank, T = _plan(cfg)
    nc = bacc.Bacc("TRN2", target_bir_lowering=False, debug=False,
                   num_devices=N_CORES)

    B, D0, OBINS = cfg["B"], cfg["D0"], cfg["OBINS"]
    b_core = T * P

    xs = nc.dram_tensor("xs", [b_core, D0], F32, kind="ExternalInput")
    wsh = nc.dram_tensor("wsh", [per_rank, P, OBW], F32, kind="ExternalInput")
    mask_mean = nc.dram_tensor("mask_mean", [4, P, per_rank], F32, kind="ExternalInput")
    mask_bin = nc.dram_tensor("mask_bin", [4, P, per_rank], F32, kind="ExternalInput")
    mz_out = nc.dram_tensor("mz", [b_core, OBINS], F32, kind="ExternalOutput")
    ii_out = nc.dram_tensor("ii", [b_core, OBINS], F32, kind="ExternalOutput")

    rc0 = _rank_chunks(layers, 0)  # shard layout identical on every rank

    with tile.TileContext(nc) as tc:
        with (
            tc.tile_pool(name="ybig", bufs=4) as ypool,        # [128,4096] f32 slots
            tc.tile_pool(name="xqT", bufs=5) as xqTpool,       # [128,32,128] bf16
            tc.tile_pool(name="xqT0", bufs=4) as xqT0pool,     # [128,n_ic0,128] bf16
            tc.tile_pool(name="wp", bufs=2) as wpool,          # [128,8192] bf16
            tc.tile_pool(name="xqn", bufs=2) as xqnpool,       # [128,4096] bf16
            tc.tile_pool(name="sg", bufs=3) as sgpool,         # [128,512] f32
            tc.tile_pool(name="u", bufs=3) as upool,           # [128,512] f32
            tc.tile_pool(name="outr", bufs=2) as outpool,      # [128,OBINS] f32
            tc.tile_pool(name="small", bufs=1) as small,
            tc.tile_pool(name="psum", bufs=8, space="PSUM") as psum,
            tc.tile_pool(name="dram", bufs=1, space="DRAM") as dram,
        ):
          try:
            # ---------------- DRAM scratch ----------------
            stage = []
            image = []
            for L in layers:
                stage.append(dram.tile([L["per_rank"] * P, OBW], BF16,
                                       tag=f"stage{L['li']}", name=f"stage{L['li']}"))
                image.append(dram.tile([L["n_ch"] * P, OBW], BF16,
                                       tag=f"image{L['li']}", name=f"image{L['li']}",
                                       addr_space="Shared"))
            ar_in = dram.tile([P, 4], F32, tag="ar_in")
            ar_out = dram.tile([P, 4], F32, tag="ar_out", addr_space="Shared")

            # ---------------- Stage A: input activation quant ----------------
            n_ic0 = layers[0]["n_ic"]
            xqT_cur = []
            am0s = []
            for t in range(T):
                xt = ypool.tile([P, D0], F32, tag="y")
                nc.sync.dma_start(xt[:], xs[t * P:(t + 1) * P, :])
                am = small.tile([P, 1], F32, tag=f"am0_{t}")
                nc.vector.tensor_reduce(am[:], xt[:], mybir.AxisListType.X,
                                        OP.max, apply_absolute_value=True)
                nc.vector.tensor_scalar(am[:], am[:], float(EPS), None, OP.max)
                sc = small.tile([P, 1], F32, tag=f"s0_{t}")
                nc.vector.tensor_scalar(sc[:], am[:], 1.0 / 127.0, None, OP.mult)
                nc.vector.reciprocal(sc[:], sc[:])
                xq0 = xqnpool.tile([P, D0], BF16, tag="xqn")
                for ch in range(D0 // OBW):
                    uu = upool.tile([P, OBW], F32, tag="u")
                    nc.scalar.activation(uu[:], xt[:, ch * OBW:(ch + 1) * OBW],
                                         AF.Copy, bias=MAGIC, scale=sc[:])
                    nc.vector.tensor_scalar(xq0[:, ch * OBW:(ch + 1) * OBW],
                                            uu[:], MAGIC, None, OP.subtract)
                xqT0 = xqT0pool.tile([P, n_ic0, P], BF16, tag="xqT0")
                nc.scalar.dma_start_transpose(xqT0[:], xq0[:])
                xqT_cur.append(xqT0)
                am0s.append(am)

            # ---------------- Stage B: weight scales ----------------
            def _stop(label):
                if kstop == label:
                    raise _StopBuild()
            _stop("A")
            partials = small.tile([P, per_rank], F32, tag="partials")
            shard_runs = _runs(list(range(per_rank)))
            for run in shard_runs:
                j0, rl = run[0], len(run)
                wrun = ypool.tile([P, rl, OBW], F32, tag="y", name=f"wrun_{j0}")
                nc.sync.dma_start(
                    wrun[:], wsh[j0:j0 + rl, :, :].rearrange("j p f -> p j f"))
                for k in range(rl):
                    junk = sgpool.tile([P, OBW], F32, tag="sg")
                    nc.scalar.activation(junk[:], wrun[:, k, :],
                                         AF.Abs, bias=0.0, scale=1.0,
                                         accum_out=partials[:, j0 + k:j0 + k + 1])
            _stop("B1")
            maskm_sb = small.tile([P, 4, per_rank], F32, tag="maskm")
            maskb_sb = small.tile([P, 4, per_rank], F32, tag="maskb")
            nc.sync.dma_start(maskm_sb[:], mask_mean[:].rearrange("m p j -> p m j"))
            nc.sync.dma_start(maskb_sb[:], mask_bin[:].rearrange("m p j -> p m j"))
            pm = small.tile([P, 4], F32, tag="pm")
            junk88 = small.tile([P, per_rank], F32, tag="junk88")
            for m in range(4):
                nc.vector.tensor_tensor(junk88[:], partials[:],
                                        maskm_sb[:, m, :], OP.mult)
                nc.vector.tensor_reduce(pm[:, m:m + 1], junk88[:],
                                        mybir.AxisListType.X, OP.add)
            _stop("B2")
            nc.sync.dma_start(ar_in[:], pm[:])
            nc.gpsimd.collective_compute(
                "AllReduce", OP.add,
                ins=[ar_in.opt()], outs=[ar_out.opt()],
                replica_groups=[list(range(N_CORES))])
            _stop("B3")
            pmsum = small.tile([P, 4], F32, tag="pmsum")
            nc.sync.dma_start(pmsum[:], ar_out[:])
            mean4 = small.tile([1, 4], F32, tag="mean4")
            nc.gpsimd.tensor_reduce(mean4[:], pmsum[:], mybir.AxisListType.C, OP.add)
            mw4 = small.tile([1, 4], F32, tag="mw4")
            nc.vector.tensor_scalar(mw4[:], mean4[:], float(EPS), None, OP.max)
            sw4 = small.tile([1, 4], F32, tag="sw4")
            nc.vector.reciprocal(sw4[:], mw4[:])
            mwb = small.tile([P, 4], F32, tag="mwb")
            swb = small.tile([P, 4], F32, tag="swb")
            nc.gpsimd.partition_broadcast(mwb[:], mw4[:])
            nc.gpsimd.partition_broadcast(swb[:], sw4[:])
            _stop("B4")
            s_pc = small.tile([P, per_rank], F32, tag="s_pc")
            nc.vector.memset(s_pc[:], 0.0)
            for m in range(4):
                nc.vector.scalar_tensor_tensor(
                    s_pc[:], maskb_sb[:, m, :],
                    swb[:, m:m + 1], s_pc[:], OP.mult, OP.add)
            # per-row dequant scale for layer 0
            c_cur = []
            for t in range(T):
                c0 = small.tile([P, 1], F32, tag=f"c0_{t}")
                nc.vector.scalar_tensor_tensor(c0[:], am0s[t][:], 1.0 / 127.0,
                                               mwb[:, 0:1], OP.mult, OP.mult)
                c_cur.append(c0)

            # ---------------- Stage C: quantize weights + AllGather ----------
            if kstop == "B":
                raise _StopBuild()
            jofs = 0
            for L in layers:
                li = L["li"]
                js = list(range(jofs, jofs + L["per_rank"]))
                for run in _runs(js):
                    j0, rl = run[0], len(run)
                    wrun = ypool.tile([P, rl, OBW], F32, tag="y", name=f"wrunq_{j0}")
                    nc.sync.dma_start(
                        wrun[:], wsh[j0:j0 + rl, :, :].rearrange("j p f -> p j f"))
                    qrun = xqnpool.tile([P, rl, OBW], BF16, tag="xqn", name=f"qrun_{j0}")
                    for k in range(rl):
                        j = j0 + k
                        uu = upool.tile([P, OBW], F32, tag="u")
                        nc.scalar.activation(uu[:], wrun[:, k, :],
                                             AF.Copy, bias=MAGIC,
                                             scale=s_pc[:, j:j + 1])
                        vv = sgpool.tile([P, OBW], F32, tag="sg")
                        nc.vector.tensor_scalar(vv[:], uu[:], MAGIC, 1.0,
                                                OP.subtract, OP.min)
                        nc.vector.tensor_scalar(qrun[:, k, :],
                                                vv[:], -1.0, None, OP.max)
                    r0 = (j0 - jofs) * P
                    nc.sync.dma_start(
                        stage[li][r0:r0 + rl * P, :].rearrange(
                            "(j p) f -> p j f", p=P),
                        qrun[:])
                jofs += L["per_rank"]
                nc.gpsimd.collective_compute(
                    "AllGather", OP.bypass,
                    ins=[stage[li].opt()], outs=[image[li].opt()],
                    replica_groups=[list(range(N_CORES))])

            # ---------------- Stage D: main pass ----------------
            d_layers = [] if kstop in ("A", "B", "C") else (
                layers[:int(kstop[1:]) + 1] if kstop.startswith("D") else layers)
            for L in d_layers:
                li, n_ic, n_ob = L["li"], L["n_ic"], L["n_ob"]
                panel_ic, n_panels = L["panel_ic"], L["n_panels"]
                dout, dreal = L["dout"], L["dreal"]
                is_last = (li == 3)

                ys = [ypool.tile([P, dreal], F32, tag="y", name=f"y{li}_{t}") for t in range(T)]
                bns = [small.tile([P, n_ob * 6], F32, tag=f"bn{t}", name=f"bn{li}_{t}")
                       for t in range(T)] if not is_last else None

                for ob in range(n_ob):
                    ow = L["ob_w"][ob]
                    ps = [psum.tile([P, OBW], F32, tag="ps", name=f"ps{li}_{ob}_{t}") for t in range(T)]
                    for panel in range(n_panels):
                        wp = wpool.tile([P, panel_ic, OBW], BF16, tag="wp",
                                        name=f"wp{li}_{ob}_{panel}")
                        r0 = (ob * n_ic + panel * panel_ic) * P
                        nc.sync.dma_start(
                            wp[:], image[li][r0:r0 + panel_ic * P, :].rearrange(
                                "(c p) f -> p c f", p=P))
                        for t in range(T):
                            for cc in range(panel_ic):
                                c = panel * panel_ic + cc
                                nc.tensor.matmul(
                                    ps[t][:], xqT_cur[t][:, c, :],
                                    wp[:, cc, :],
                                    start=(c == 0), stop=(c == n_ic - 1))
                    for t in range(T):
                        dst = ys[t][:, ob * OBW:ob * OBW + ow]
                        if not is_last:
                            nc.scalar.activation(dst, ps[t][:, :ow], AF.Copy,
                                                 bias=0.0, scale=c_cur[t][:])
                            nc.vector.bn_stats(bns[t][:, ob * 6:(ob + 1) * 6], dst)
                        else:
                            nc.scalar.activation(dst, ps[t][:, :ow], AF.Sigmoid,
                                                 bias=0.0, scale=c_cur[t][:])

                if is_last:
                    for t in range(T):
                        mzt = outpool.tile([P, OBINS], F32, tag="outr")
                        nc.vector.tensor_scalar(mzt[:], ys[t][:, 0:OBINS],
                                                float(OBINS - 1), 1.0,
                                                OP.mult, OP.add)
                        nc.scalar.dma_start(mz_out[t * P:(t + 1) * P, :], mzt[:])
                        iit = outpool.tile([P, OBINS], F32, tag="outr")
                        nc.vector.tensor_scalar(iit[:], ys[t][:, OBINS:2 * OBINS],
                                                100.0, None, OP.mult)
                        nc.scalar.dma_start(ii_out[t * P:(t + 1) * P, :], iit[:])
                    continue

                # ---- tail: LN + SiLU + act quant + transpose ----
                n_ic_next = layers[li + 1]["n_ic"]
                xqT_next = []
                c_next = []
                for t in range(T):
                    mv = small.tile([P, 2], F32, tag="mv")
                    nc.vector.bn_aggr(mv[:], bns[t][:])
                    v = small.tile([P, 1], F32, tag="vvar")
                    nc.vector.tensor_scalar(v[:], mv[:, 1:2], float(EPS), None, OP.add)
                    istd = _rsqrt_newton(nc, small, v[:])
                    nmi = small.tile([P, 1], F32, tag="nmi")
                    nc.vector.scalar_tensor_tensor(nmi[:], mv[:, 0:1], -1.0,
                                                   istd[:], OP.mult, OP.mult)
                    # z = (y - mu) * istd, in place
                    nc.scalar.activation(ys[t][:], ys[t][:], AF.Identity,
                                         bias=nmi[:], scale=istd[:])
                    amsl = small.tile([P, 8], F32, tag="amsl")
                    n_chk = dout // OBW
                    for ch in range(n_chk):
                        sl = ys[t][:, ch * OBW:(ch + 1) * OBW]
                        sg = sgpool.tile([P, OBW], F32, tag="sg")
                        nc.scalar.activation(sg[:], sl, AF.Sigmoid,
                                             bias=0.0, scale=1.0)
                        nc.vector.tensor_tensor(sl, sl, sg[:], OP.mult)
                        nc.vector.tensor_reduce(amsl[:, ch:ch + 1], sl,
                                                mybir.AxisListType.X, OP.max,
                                                apply_absolute_value=True)
                    am = small.tile([P, 1], F32, tag="amn")
                    nc.vector.tensor_reduce(am[:], amsl[:, :n_chk],
                                            mybir.AxisListType.X, OP.max)
                    nc.vector.tensor_scalar(am[:], am[:],
                                            float(EPS), None, OP.max)
                    sc = small.tile([P, 1], F32, tag="scn")
                    nc.vector.tensor_scalar(sc[:], am[:], 1.0 / 127.0, None, OP.mult)
                    nc.vector.reciprocal(sc[:], sc[:])
                    cn = small.tile([P, 1], F32, tag=f"c{li + 1}_{t}")
                    nc.vector.scalar_tensor_tensor(cn[:], am[:], 1.0 / 127.0,
                                                   mwb[:, li + 1:li + 2],
                                                   OP.mult, OP.mult)
                    c_next.append(cn)
                    xqn = xqnpool.tile([P, dout], BF16, tag="xqn")
                    for ch in range(n_chk):
                        uu = upool.tile([P, OBW], F32, tag="u")
                        nc.scalar.activation(uu[:], ys[t][:, ch * OBW:(ch + 1) * OBW],
                                             AF.Copy, bias=MAGIC, scale=sc[:])
                        nc.vector.tensor_scalar(xqn[:, ch * OBW:(ch + 1) * OBW],
                                                uu[:], MAGIC, None, OP.subtract)
                    xT = xqTpool.tile([P, n_ic_next, P], BF16, tag="xqT")
                    nc.scalar.dma_start_transpose(xT[:], xqn[:])
                    xqT_next.append(xT)
                xqT_cur = xqT_next
                c_cur = c_next

          except _StopBuild:
            pass
    nc.compile()
    return nc


class _StopBuild(Exception):
    pass


def prepare_inputs(cfg, x, W0, W1, W2, W3):
    """Host-side sharding: per-core input maps."""
    layers, total_ch, per_rank, T = _plan(cfg)
    b_core = T * P
    Ws = [np.asarray(W0), np.asarray(W1), np.asarray(W2), np.asarray(W3)]
    # transposed (and, for the last layer, output-padded) weights: WT[i, o]
    WTs = []
    for L, W in zip(layers, Ws):
        WT = np.zeros((L["din"], L["dout"]), dtype=np.float32)
        WT[:, :L["dreal"]] = W.T
        WTs.append(WT)

    in_maps = []
    x = np.asarray(x, dtype=np.float32)
    for r in range(N_CORES):
        chunks = _rank_chunks(layers, r)
        wsh = np.empty((per_rank, P, OBW), dtype=np.float32)
        mask_mean = np.zeros((4, P, per_rank), dtype=np.float32)
        mask_bin = np.zeros((4, P, per_rank), dtype=np.float32)
        for j, (li, g) in enumerate(chunks):
            L = layers[li]
            ob, ic = divmod(g, L["n_ic"])
            wsh[j] = WTs[li][ic * P:(ic + 1) * P, ob * OBW:(ob + 1) * OBW]
            mask_bin[li, :, j] = 1.0
            mask_mean[li, :, j] = 1.0 / layers[li]["numel"]
        in_maps.append(dict(
            xs=np.ascontiguousarray(x[r * b_core:(r + 1) * b_core]),
            wsh=wsh, mask_mean=mask_mean, mask_bin=mask_bin,
        ))
    return in_maps


_NC_CACHE = {}


def _get_nc(cfg_key):
    if cfg_key not in _NC_CACHE:
        _NC_CACHE[cfg_key] = build(dict(cfg_key))
    return _NC_CACHE[cfg_key]


def run(cfg, x, W0, W1, W2, W3, trace=False):
    layers, total_ch, per_rank, T = _plan(cfg)
    b_core = T * P
    nc = _get_nc(tuple(sorted(cfg.items())))
    in_maps = prepare_inputs(cfg, x, W0, W1, W2, W3)
    res = run_bass_kernel_spmd(nc, in_maps, core_ids=list(range(N_CORES)),
                               trace=trace)
    mz = np.concatenate([res.results[r]["mz"] for r in range(N_CORES)], axis=0)
    ii = np.concatenate([res.results[r]["ii"] for r in range(N_CORES)], axis=0)
    return (mz, ii), res


def kernel(x, W0, W1, W2, W3, g0, b0, g1, b1, g2, b2):
    """Full-input entry point. g/b are identity (ones/zeros) in this problem's
    setup; LayerNorm affine is a no-op and is validated here."""
    for g in (g0, g1, g2):
        assert np.allclose(np.asarray(g), 1.0), "non-identity LN gain unsupported"
    for b in (b0, b1, b2):
        assert np.allclose(np.asarray(b), 0.0), "non-zero LN bias unsupported"
    (mz, ii), _ = run(FULL_CFG, x, W0, W1, W2, W3, trace=False)
    return (mz, ii)
